# revision 3
# baseline (speedup 1.0000x reference)
"""CapsNet forward pass on 8 Trainium2 NeuronCores (Bass/Tile).

Distribution strategy:
- conv (9x9, as matmul over host-im2col'd patches) replicated on all cores
- PrimaryCaps FC ([9216, 102400] weight, 3.77GB f32) sharded column-wise
  (output features): each core computes its 1152 features for all 32
  batches, streaming its 471MB weight shard from HBM. This is the
  memory-bound bulk of the model; the matmul runs in float32r (full-rate
  PE) so the kernel stays HBM-bound.
- AllToAll redistributes pc so each core holds its 4 batches x all 9216
  features; squash/routing/decoder then run data-parallel over batch.
- host reassembles the 6 reference outputs.

kernel(**inputs) takes the full (unsharded) inputs and returns the full
outputs as the reference does. Self-contained: imports only environment
packages (numpy/jax/concourse).
"""

import os
import time

import numpy as np
import ml_dtypes

import concourse.bacc as bacc
import concourse.mybir as mybir
import concourse.tile as tile

N_CORES = 8
B = 32
B_LOC = B // N_CORES            # 4 batches per core
NFEAT = 9216                    # fc output features
NSH = NFEAT // N_CORES          # 1152 per core
NCAPS = 1152                    # primary capsules
CAPS_SH = NCAPS // N_CORES      # 144 capsules per source core in AllToAll
K = 102400                      # fc contraction
NCHUNK = K // 128               # 800
NPOS = 400                      # conv output positions (20x20)
OC = 256                        # conv out channels
NT = 9                          # capsule tiles (1152 = 9*128)
ND = 160                        # 10 digit caps x 16 dims
EPS = 1e-8

f32 = mybir.dt.float32
AF = mybir.ActivationFunctionType
ALU = mybir.AluOpType

# FC matmul dtype: "f32r" (tf32-like, full PE rate), "f32", "bf16"
VARIANT = os.environ.get("CAPSNET_VARIANT", "f32r")
DMA_BATCH = int(os.environ.get("CAPSNET_DMA_BATCH", "2"))


def _fc_dt():
    if VARIANT == "bf16":
        return mybir.dt.bfloat16
    if VARIANT == "f32r":
        return mybir.dt.float32r
    return f32


def _bcast(ap, shape):
    return ap.broadcast_to(shape)


def build_program(nchunk=NCHUNK):
    wdt = _fc_dt()
    nc = bacc.Bacc("TRN2", target_bir_lowering=False, debug=False,
                   num_devices=N_CORES)

    # ---- inputs (per core) ----
    w_in = nc.dram_tensor("w", [128 * nchunk, NSH], wdt, kind="ExternalInput")
    patches_in = nc.dram_tensor("patches", [81, NPOS * B], f32,
                                kind="ExternalInput")
    cw_in = nc.dram_tensor("cw", [81, OC], f32, kind="ExternalInput")
    cb_in = nc.dram_tensor("cb", [128, 2], f32, kind="ExternalInput")
    fcb_in = nc.dram_tensor("fcb", [1, NSH], wdt, kind="ExternalInput")
    ones_in = nc.dram_tensor("ones", [1, B], wdt, kind="ExternalInput")
    onesf_in = nc.dram_tensor("onesf", [128, 128], f32, kind="ExternalInput")
    ident_in = nc.dram_tensor("ident", [8, 8], f32, kind="ExternalInput")
    wr_in = nc.dram_tensor("wr", [NCAPS, 8 * ND], f32, kind="ExternalInput")
    d1w_in = nc.dram_tensor("d1w", [160, 512], f32, kind="ExternalInput")
    d1b_in = nc.dram_tensor("d1b", [1, 512], f32, kind="ExternalInput")
    d2w_in = nc.dram_tensor("d2w", [512, 1024], f32, kind="ExternalInput")
    d2b_in = nc.dram_tensor("d2b", [1, 1024], f32, kind="ExternalInput")
    d3w_in = nc.dram_tensor("d3w", [1024, 784], f32, kind="ExternalInput")
    d3b_in = nc.dram_tensor("d3b", [1, 784], f32, kind="ExternalInput")

    # ---- outputs (per core) ----
    caps_out = nc.dram_tensor("caps_o", [128, NT, B_LOC, 8], f32,
                              kind="ExternalOutput")
    c_out = nc.dram_tensor("c_o", [128, NT, B_LOC, 10], f32,
                           kind="ExternalOutput")
    b_out = nc.dram_tensor("b_o", [128, NT, B_LOC, 10], f32,
                           kind="ExternalOutput")
    digit_out = nc.dram_tensor("digit_o", [1, B_LOC * ND], f32,
                               kind="ExternalOutput")
    logits_out = nc.dram_tensor("logits_o", [1, B_LOC * 10], f32,
                                kind="ExternalOutput")
    recon_out = nc.dram_tensor("recon_o", [B_LOC, 784], f32,
                               kind="ExternalOutput")

    with tile.TileContext(nc) as tc:
        with (
            tc.tile_pool(name="persist", bufs=1) as pers,
            tc.tile_pool(name="dram", bufs=1, space="DRAM") as dram,
        ):
            ones_sb = pers.tile([1, B], wdt, name="ones_sb")
            nc.sync.dma_start(ones_sb[:], ones_in[:])
            onesf_sb = pers.tile([128, 128], f32, name="onesf_sb")
            nc.sync.dma_start(onesf_sb[:], onesf_in[:])
            ident_sb = pers.tile([8, 8], f32, name="ident_sb")
            nc.sync.dma_start(ident_sb[:], ident_in[:])
            fcb_sb = pers.tile([1, NSH], wdt, name="fcb_sb")
            nc.sync.dma_start(fcb_sb[:], fcb_in[:])
            pc_sb = pers.tile([B, NSH], f32, name="pc_sb")

            # ================= conv + relu, then FC =================
            adt = wdt
            with (
                tc.tile_pool(name="convp", bufs=1) as convp,
                tc.tile_pool(name="actp", bufs=1) as actp,
                tc.tile_pool(name="cpsum", bufs=4, space="PSUM") as cpsum,
            ):
                patches_sb = convp.tile([81, NPOS * B], f32, name="patches_sb")
                nc.sync.dma_start(patches_sb[:], patches_in[:])
                cw_sb = convp.tile([81, OC], f32, name="cw_sb")
                nc.sync.dma_start(cw_sb[:], cw_in[:])
                cb_sb = convp.tile([128, 2], f32, name="cb_sb")
                nc.sync.dma_start(cb_sb[:], cb_in[:])

                # act[half][p=oc_local, (pos, b)]
                act0 = actp.tile([128, NPOS * B], adt, name="act0")
                act1 = actp.tile([128, NPOS * B], adt, name="act1")
                act = [act0, act1]
                CCH = 512
                for half in range(2):
                    for ch in range(NPOS * B // CCH):
                        cps = cpsum.tile([128, CCH], f32, tag="cps",
                                         name="cps")
                        nc.tensor.matmul(
                            cps[:],
                            cw_sb[:, half * 128:(half + 1) * 128],
                            patches_sb[:, ch * CCH:(ch + 1) * CCH],
                            start=True, stop=True,
                        )
                        nc.scalar.activation(
                            act[half][:, ch * CCH:(ch + 1) * CCH], cps[:],
                            AF.Relu, bias=cb_sb[:, half:half + 1], scale=1.0,
                        )

                NB = 3
                NSUB = NSH // NB  # 384
                with (
                    tc.tile_pool(name="wpool", bufs=3) as wpool,
                    tc.tile_pool(name="fcps", bufs=1, space="PSUM") as fcps,
                ):
                    banks = [
                        fcps.tile([B, NSUB], f32, name=f"fbank{i}",
                                  tag=f"fbank{i}")
                        for i in range(NB)
                    ]
                    # bias seeds the accumulation groups
                    for b in range(NB):
                        nc.tensor.matmul(
                            banks[b][:], ones_sb[:, :],
                            fcb_sb[:, b * NSUB:(b + 1) * NSUB],
                            start=True, stop=False,
                        )
                    nsteps = nchunk // DMA_BATCH
                    for s in range(nsteps):
                        wt = wpool.tile([128, DMA_BATCH * NSH], wdt, tag="w",
                                        name="wt")
                        src = w_in[s * 128 * DMA_BATCH:
                                   (s + 1) * 128 * DMA_BATCH, :]
                        nc.sync.dma_start(
                            wt[:].rearrange("p (c n) -> p c n", c=DMA_BATCH),
                            src.rearrange("(c p) n -> p c n", p=128),
                        )
                        for cb_ in range(DMA_BATCH):
                            kc = s * DMA_BATCH + cb_
                            pos, half = kc // 2, kc % 2
                            lhsT = act[half][:, pos * B:(pos + 1) * B]
                            for b in range(NB):
                                nc.tensor.matmul(
                                    banks[b][:], lhsT,
                                    wt[:, cb_ * NSH + b * NSUB:
                                       cb_ * NSH + (b + 1) * NSUB],
                                    start=False, stop=(kc == nchunk - 1),
                                )
                    for b in range(NB):
                        nc.vector.tensor_copy(
                            pc_sb[:, b * NSUB:(b + 1) * NSUB], banks[b][:])

            # ================= AllToAll =================
            pc_bounce = dram.tile([B, NSH], f32, name="pc_bounce")
            pc_mine = dram.tile([B, NSH], f32, name="pc_mine")
            nc.sync.dma_start(pc_bounce[:], pc_sb[:])
            nc.gpsimd.collective_compute(
                "AllToAll", ALU.bypass,
                replica_groups=[list(range(N_CORES))],
                ins=[pc_bounce.opt()], outs=[pc_mine.opt()],
            )
            # pc_mine row (cc*4 + bl) = global batch (my 4) bl,
            # features cc*1152..(cc+1)*1152

            # ================= routing =================
            with (
                tc.tile_pool(name="routep", bufs=1) as rp,
                tc.tile_pool(name="wrp", bufs=2) as wrp,
                tc.tile_pool(name="tmpp", bufs=2) as tmpp,
                tc.tile_pool(name="rps", bufs=1, space="PSUM") as rps,
                tc.tile_pool(name="decp", bufs=1) as decp,
            ):
                # --- gather pc into capsule layout [p, t, (b, i)] ---
                pcT = rp.tile([128, NT, B_LOC * 8], f32, name="pcT")
                for t in range(NT):
                    p0, p1 = t * 128, t * 128 + 128
                    for cc in range(p0 // CAPS_SH, (p1 - 1) // CAPS_SH + 1):
                        s0, s1 = max(p0, cc * CAPS_SH), min(p1, (cc + 1) * CAPS_SH)
                        src = pc_mine[cc * B_LOC:(cc + 1) * B_LOC,
                                      (s0 - cc * CAPS_SH) * 8:
                                      (s1 - cc * CAPS_SH) * 8]
                        src = src.rearrange("b (p i) -> p b i", i=8)
                        dst = pcT[s0 - p0:s1 - p0, t, :].rearrange(
                            "p (b i) -> p b i", i=8)
                        nc.sync.dma_start(dst, src)

                # --- squash -> caps ---
                caps = rp.tile([128, NT, B_LOC * 8], f32, name="caps")
                n2 = rp.tile([128, B_LOC], f32, name="n2")
                f1 = rp.tile([128, B_LOC], f32, name="f1")
                f2 = rp.tile([128, B_LOC], f32, name="f2")
                sc = rp.tile([128, B_LOC], f32, name="sc")
                tmp_s = rp.tile([128, B_LOC], f32, name="tmp_s")
                for t in range(NT):
                    pct = pcT[:, t, :].rearrange("p (b i) -> p b i", i=8)
                    sq = tmpp.tile([128, B_LOC * 8], f32, tag="sqtmp",
                                   name="sq")
                    nc.scalar.activation(sq[:], pcT[:, t, :], AF.Square)
                    nc.vector.reduce_sum(
                        n2[:], sq[:].rearrange("p (b i) -> p b i", i=8),
                        axis=mybir.AxisListType.X)
                    nc.scalar.activation(tmp_s[:], n2[:], AF.Sqrt)
                    nc.vector.reciprocal(f1[:], tmp_s[:])
                    nc.scalar.activation(tmp_s[:], n2[:], AF.Copy,
                                         bias=1.0, scale=1.0)
                    nc.vector.reciprocal(sc[:], tmp_s[:])
                    nc.vector.tensor_scalar(sc[:], sc[:], -1.0, 1.0,
                                            ALU.mult, ALU.add)
                    nc.vector.tensor_mul(f1[:], f1[:], sc[:])
                    nc.vector.tensor_scalar(f2[:], sc[:], EPS, None, ALU.mult)
                    ct_ = caps[:, t, :].rearrange("p (b i) -> p b i", i=8)
                    nc.vector.tensor_tensor(
                        ct_, pct, _bcast(f1[:].unsqueeze(2), [128, B_LOC, 8]),
                        ALU.mult)
                    nc.vector.tensor_tensor(
                        ct_, ct_, _bcast(f2[:].unsqueeze(2), [128, B_LOC, 8]),
                        ALU.add)
                nc.sync.dma_start(
                    caps_out[:, :, :, :].rearrange("p t b i -> p t (b i)"),
                    caps[:])

                # --- u_hat[p, t, (b, n, d)] ---
                u_hat = rp.tile([128, NT, B_LOC * ND], f32, name="u_hat")
                for t in range(NT):
                    wr_t = wrp.tile([128, 8 * ND], f32, tag="wr", name="wr_t")
                    nc.sync.dma_start(wr_t[:], wr_in[t * 128:(t + 1) * 128, :])
                    mu = tmpp.tile([128, B_LOC * ND * 8], f32, tag="mu",
                                   name="mu")
                    wr_ap = wr_t[:].rearrange("p (i nd) -> p nd i", i=8)
                    wr_ap = _bcast(wr_ap.unsqueeze(1), [128, B_LOC, ND, 8])
                    cap_ap = caps[:, t, :].rearrange("p (b i) -> p b i", i=8)
                    cap_ap = _bcast(cap_ap.unsqueeze(2), [128, B_LOC, ND, 8])
                    mu_ap = mu[:].rearrange("p (b nd i) -> p b nd i",
                                            nd=ND, i=8)
                    nc.vector.tensor_tensor(mu_ap, wr_ap, cap_ap, ALU.mult)
                    nc.vector.reduce_sum(
                        u_hat[:, t, :],
                        mu[:].rearrange("p (bnd i) -> p bnd i", i=8),
                        axis=mybir.AxisListType.X)

                # --- routing state ---
                bst = rp.tile([128, NT, B_LOC * 10], f32, name="bst")
                cst = rp.tile([128, NT, B_LOC * 10], f32, name="cst")
                nc.vector.memset(bst[:], 0.0)
                mx = rp.tile([128, B_LOC], f32, name="mx")
                sm = rp.tile([128, B_LOC], f32, name="sm")
                v_sb = rp.tile([1, B_LOC * ND], f32, name="v_sb")
                vb_sb = rp.tile([128, B_LOC * ND], f32, name="vb_sb")
                sn2 = rp.tile([1, B_LOC * 10], f32, name="sn2")
                sf1 = rp.tile([1, B_LOC * 10], f32, name="sf1")
                sf2 = rp.tile([1, B_LOC * 10], f32, name="sf2")
                ssc = rp.tile([1, B_LOC * 10], f32, name="ssc")
                stmp = rp.tile([1, B_LOC * 10], f32, name="stmp")

                def softmax_c():
                    for t in range(NT):
                        bt = bst[:, t, :].rearrange("p (b n) -> p b n", n=10)
                        ct = cst[:, t, :].rearrange("p (b n) -> p b n", n=10)
                        nc.vector.reduce_max(mx[:], bt,
                                             axis=mybir.AxisListType.X)
                        nc.vector.tensor_tensor(
                            ct, bt,
                            _bcast(mx[:].unsqueeze(2), [128, B_LOC, 10]),
                            ALU.subtract)
                        nc.scalar.activation(cst[:, t, :], cst[:, t, :],
                                             AF.Exp)
                        nc.vector.reduce_sum(sm[:], ct,
                                             axis=mybir.AxisListType.X)
                        nc.vector.reciprocal(sm[:], sm[:])
                        nc.vector.tensor_tensor(
                            ct, ct,
                            _bcast(sm[:].unsqueeze(2), [128, B_LOC, 10]),
                            ALU.mult)

                def compute_s():
                    s_ps = [rps.tile([1, 320], f32, tag=f"sps{h}",
                                     name=f"s_ps{h}") for h in range(2)]
                    for t in range(NT):
                        m2 = tmpp.tile([128, B_LOC * ND], f32, tag="m2",
                                       name="m2")
                        m2_ap = m2[:].rearrange("p (b n d) -> p b n d",
                                                n=10, d=16)
                        ct = cst[:, t, :].rearrange("p (b n) -> p b n", n=10)
                        uh = u_hat[:, t, :].rearrange("p (b n d) -> p b n d",
                                                      n=10, d=16)
                        nc.vector.tensor_tensor(
                            m2_ap, uh,
                            _bcast(ct.unsqueeze(3), [128, B_LOC, 10, 16]),
                            ALU.mult)
                        for h in range(2):
                            nc.tensor.matmul(
                                s_ps[h][:], onesf_sb[:, :1],
                                m2[:, h * 320:(h + 1) * 320],
                                start=(t == 0), stop=(t == NT - 1),
                            )
                    return s_ps

                def squash640(dst, s_ps):
                    for h in range(2):
                        nc.vector.tensor_copy(
                            dst[:, h * 320:(h + 1) * 320], s_ps[h][:])
                    sq6 = tmpp.tile([1, B_LOC * ND], f32, tag="sq6",
                                    name="sq6")
                    nc.scalar.activation(sq6[:], dst[:], AF.Square)
                    nc.vector.reduce_sum(
                        sn2[:], sq6[:].rearrange("q (bn d) -> q bn d", d=16),
                        axis=mybir.AxisListType.X)
                    nc.scalar.activation(stmp[:], sn2[:], AF.Sqrt)
                    nc.vector.reciprocal(sf1[:], stmp[:])
                    nc.scalar.activation(stmp[:], sn2[:], AF.Copy,
                                         bias=1.0, scale=1.0)
                    nc.vector.reciprocal(ssc[:], stmp[:])
                    nc.vector.tensor_scalar(ssc[:], ssc[:], -1.0, 1.0,
                                            ALU.mult, ALU.add)
                    nc.vector.tensor_mul(sf1[:], sf1[:], ssc[:])
                    nc.vector.tensor_scalar(sf2[:], ssc[:], EPS, None,
                                            ALU.mult)
                    dstv = dst[:].rearrange("q (bn d) -> q bn d", d=16)
                    nc.vector.tensor_tensor(
                        dstv, dstv,
                        _bcast(sf1[:].unsqueeze(2), [1, B_LOC * 10, 16]),
                        ALU.mult)
                    nc.vector.tensor_tensor(
                        dstv, dstv,
                        _bcast(sf2[:].unsqueeze(2), [1, B_LOC * 10, 16]),
                        ALU.add)

                for _ in range(2):
                    softmax_c()
                    s_ps = compute_s()
                    squash640(v_sb, s_ps)
                    vb_ps = [rps.tile([128, 320], f32, tag=f"vb{h}",
                                      name=f"vb_ps{h}") for h in range(2)]
                    for h in range(2):
                        nc.tensor.matmul(
                            vb_ps[h][:], onesf_sb[:1, :],
                            v_sb[:, h * 320:(h + 1) * 320],
                            start=True, stop=True,
                        )
                        nc.vector.tensor_copy(
                            vb_sb[:, h * 320:(h + 1) * 320], vb_ps[h][:])
                    for t in range(NT):
                        m3 = tmpp.tile([128, B_LOC * ND], f32, tag="m3",
                                       name="m3")
                        nc.vector.tensor_tensor(m3[:], u_hat[:, t, :],
                                                vb_sb[:], ALU.mult)
                        uv = tmpp.tile([128, B_LOC * 10], f32, tag="uv",
                                       name="uv")
                        nc.vector.reduce_sum(
                            uv[:], m3[:].rearrange("p (bn d) -> p bn d", d=16),
                            axis=mybir.AxisListType.X)
                        nc.vector.tensor_add(bst[:, t, :], bst[:, t, :],
                                             uv[:])

                # final c (output), s, digit
                softmax_c()
                nc.sync.dma_start(
                    c_out[:, :, :, :].rearrange("p t b n -> p t (b n)"),
                    cst[:])
                nc.sync.dma_start(
                    b_out[:, :, :, :].rearrange("p t b n -> p t (b n)"),
                    bst[:])
                s_ps = compute_s()
                digit = rp.tile([1, B_LOC * ND], f32, name="digit")
                squash640(digit, s_ps)
                nc.sync.dma_start(digit_out[:], digit[:])

                # logits = ||digit|| over d
                lg = rp.tile([1, B_LOC * 10], f32, name="lg")
                sqd = tmpp.tile([1, B_LOC * ND], f32, tag="sqd", name="sqd")
                nc.scalar.activation(sqd[:], digit[:], AF.Square)
                nc.vector.reduce_sum(
                    lg[:], sqd[:].rearrange("q (bn d) -> q bn d", d=16),
                    axis=mybir.AxisListType.X)
                nc.scalar.activation(lg[:], lg[:], AF.Sqrt)
                nc.sync.dma_start(logits_out[:], lg[:])

                # one-hot mask = (logits == rowmax); masked = digit * mask
                lmx = rp.tile([1, B_LOC], f32, name="lmx")
                nc.vector.reduce_max(
                    lmx[:], lg[:].rearrange("q (b n) -> q b n", n=10),
                    axis=mybir.AxisListType.X)
                msk = rp.tile([1, B_LOC * 10], f32, name="msk")
                nc.vector.tensor_tensor(
                    msk[:].rearrange("q (b n) -> q b n", n=10),
                    lg[:].rearrange("q (b n) -> q b n", n=10),
                    _bcast(lmx[:].unsqueeze(2), [1, B_LOC, 10]),
                    ALU.is_equal)
                masked = rp.tile([1, B_LOC * ND], f32, name="masked")
                nc.vector.tensor_tensor(
                    masked[:].rearrange("q (b n d) -> q b n d", n=10, d=16),
                    digit[:].rearrange("q (b n d) -> q b n d", n=10, d=16),
                    _bcast(msk[:].rearrange("q (b n) -> q b n", n=10)
                           .unsqueeze(3), [1, B_LOC, 10, 16]),
                    ALU.mult)

                # ---- decoder ----
                d1w_sb = decp.tile([80, 2, 512], f32, name="d1w_sb")
                nc.sync.dma_start(
                    d1w_sb[:, :, :],
                    d1w_in[:, :].rearrange("(h p) n -> p h n", p=80))
                d1b_sb = decp.tile([1, 512], f32, name="d1b_sb")
                nc.sync.dma_start(d1b_sb[:], d1b_in[:])
                d2w_sb = decp.tile([128, 4, 1024], f32, name="d2w_sb")
                nc.sync.dma_start(
                    d2w_sb[:, :, :],
                    d2w_in[:, :].rearrange("(j p) n -> p j n", p=128))
                d2b_sb = decp.tile([1, 1024], f32, name="d2b_sb")
                nc.sync.dma_start(d2b_sb[:], d2b_in[:])
                d3w_sb = decp.tile([128, 8, 784], f32, name="d3w_sb")
                nc.sync.dma_start(
                    d3w_sb[:, :, :],
                    d3w_in[:, :].rearrange("(j p) n -> p j n", p=128))
                d3b_sb = decp.tile([1, 784], f32, name="d3b_sb")
                nc.sync.dma_start(d3b_sb[:], d3b_in[:])

                # masked2d [4, 160] <- masked [1, (b nd)]
                m2d = decp.tile([B_LOC, ND], f32, name="m2d")
                nc.sync.dma_start(
                    m2d[:],
                    masked[:].rearrange("q (b nd) -> q b nd", b=B_LOC))
                # maskedT [80, 4] x2 via PE transpose
                mT = decp.tile([80, 2, B_LOC], f32, name="mT")
                for h in range(2):
                    tp = rps.tile([128, B_LOC], f32, tag="tp", name="tp")
                    nc.tensor.transpose(
                        tp[:80, :], m2d[:, h * 80:(h + 1) * 80],
                        ident_sb[:B_LOC, :B_LOC])
                    nc.vector.tensor_copy(mT[:, h, :], tp[:80, :])

                # d1: h1 [4, 512]
                h1ps = rps.tile([B_LOC, 512], f32, name="h1ps", tag="decps")
                nc.tensor.matmul(h1ps[:], onesf_sb[:1, :B_LOC], d1b_sb[:],
                                 start=True, stop=False)
                for h in range(2):
                    nc.tensor.matmul(h1ps[:], mT[:, h, :], d1w_sb[:, h, :],
                                     start=False, stop=(h == 1))
                h1 = decp.tile([B_LOC, 512], f32, name="h1")
                nc.scalar.activation(h1[:], h1ps[:], AF.Relu)
                h1T = decp.tile([128, 4, B_LOC], f32, name="h1T")
                for j in range(4):
                    tp = rps.tile([128, B_LOC], f32, tag="tp", name="tp")
                    nc.tensor.transpose(
                        tp[:], h1[:, j * 128:(j + 1) * 128],
                        ident_sb[:B_LOC, :B_LOC])
                    nc.vector.tensor_copy(h1T[:, j, :], tp[:])
                # d2: h2 [4, 1024]
                h2 = decp.tile([B_LOC, 1024], f32, name="h2")
                for g in range(2):
                    h2ps = rps.tile([B_LOC, 512], f32, tag="decps",
                                    name="h2ps")
                    nc.tensor.matmul(
                        h2ps[:], onesf_sb[:1, :B_LOC],
                        d2b_sb[:, g * 512:(g + 1) * 512],
                        start=True, stop=False)
                    for j in range(4):
                        nc.tensor.matmul(
                            h2ps[:], h1T[:, j, :],
                            d2w_sb[:, j, g * 512:(g + 1) * 512],
                            start=False, stop=(j == 3))
                    nc.scalar.activation(h2[:, g * 512:(g + 1) * 512],
                                         h2ps[:], AF.Relu)
                h2T = decp.tile([128, 8, B_LOC], f32, name="h2T")
                for j in range(8):
                    tp = rps.tile([128, B_LOC], f32, tag="tp", name="tp")
                    nc.tensor.transpose(
                        tp[:], h2[:, j * 128:(j + 1) * 128],
                        ident_sb[:B_LOC, :B_LOC])
                    nc.vector.tensor_copy(h2T[:, j, :], tp[:])
                # d3: recon [4, 784] sigmoid
                recon = decp.tile([B_LOC, 784], f32, name="recon")
                for g in range(2):
                    n0, n1 = g * 512, min(784, (g + 1) * 512)
                    rps_t = rps.tile([B_LOC, 512], f32, tag="decps",
                                     name="rps_t")
                    nc.tensor.matmul(
                        rps_t[:, :n1 - n0], onesf_sb[:1, :B_LOC],
                        d3b_sb[:, n0:n1], start=True, stop=False)
                    for j in range(8):
                        nc.tensor.matmul(
                            rps_t[:, :n1 - n0], h2T[:, j, :],
                            d3w_sb[:, j, n0:n1],
                            start=False, stop=(j == 7))
                    nc.scalar.activation(recon[:, n0:n1], rps_t[:, :n1 - n0],
                                         AF.Sigmoid)
                nc.sync.dma_start(recon_out[:], recon[:])

    nc.compile()
    return nc


# ---------------- host side ----------------

def _np_wdt():
    return ml_dtypes.bfloat16 if VARIANT == "bf16" else np.float32


def prep_inputs(x, conv_w, conv_b, fc_w, fc_b, W,
                d1_w, d1_b, d2_w, d2_b, d3_w, d3_b, nchunk=NCHUNK):
    wdt_np = _np_wdt()
    x = np.ascontiguousarray(x, dtype=np.float32)

    # patchesT [81, (pos, b)]
    it = x.itemsize
    sB, _, sH, sW = (np.array(x.strides) // it)
    xs = np.lib.stride_tricks.as_strided(
        x, shape=(9, 9, 20, 20, B),
        strides=(np.array([sH, sW, sH, sW, sB]) * it))
    patches = np.ascontiguousarray(xs).reshape(81, NPOS * B)

    cw = np.ascontiguousarray(conv_w.reshape(OC, 81).T)
    cb = np.ascontiguousarray(conv_b.reshape(2, 128).T)

    # fc weight: k-order = (pos, oc); shard columns
    wT = np.ascontiguousarray(fc_w.T)                  # [(oc,pos), n]
    wT = np.ascontiguousarray(
        wT.reshape(OC, NPOS, NFEAT).transpose(1, 0, 2)).reshape(K, NFEAT)
    w_shards = [
        np.ascontiguousarray(wT[:128 * nchunk, c * NSH:(c + 1) * NSH]).astype(
            wdt_np, copy=False)
        for c in range(N_CORES)
    ]
    fcb_shards = [
        np.ascontiguousarray(
            fc_b[c * NSH:(c + 1) * NSH].reshape(1, NSH)).astype(
                wdt_np, copy=False)
        for c in range(N_CORES)
    ]

    wr = np.ascontiguousarray(
        W[0].transpose(1, 3, 0, 2)).reshape(NCAPS, 8 * ND)

    common = {
        "patches": patches,
        "cw": cw.astype(np.float32, copy=False),
        "cb": cb.astype(np.float32, copy=False),
        "ones": np.ones((1, B), dtype=wdt_np),
        "onesf": np.ones((128, 128), dtype=np.float32),
        "ident": np.eye(8, dtype=np.float32),
        "wr": wr.astype(np.float32, copy=False),
        "d1w": np.ascontiguousarray(d1_w.T),
        "d1b": np.ascontiguousarray(d1_b.reshape(1, -1)),
        "d2w": np.ascontiguousarray(d2_w.T),
        "d2b": np.ascontiguousarray(d2_b.reshape(1, -1)),
        "d3w": np.ascontiguousarray(d3_w.T),
        "d3b": np.ascontiguousarray(d3_b.reshape(1, -1)),
    }
    in_maps = []
    for c in range(N_CORES):
        m = dict(common)
        m["w"] = w_shards[c]
        m["fcb"] = fcb_shards[c]
        in_maps.append(m)
    return in_maps


def assemble(results):
    logits = np.zeros((B, 10), np.float32)
    recon = np.zeros((B, 784), np.float32)
    pcaps = np.zeros((B, NCAPS, 8), np.float32)
    dcaps = np.zeros((B, 10, 16), np.float32)
    c_full = np.zeros((B, 10, NCAPS, 1), np.float32)
    b_full = np.zeros((B, 10, NCAPS, 1), np.float32)
    for c in range(N_CORES):
        r = results[c]
        bs = slice(c * B_LOC, (c + 1) * B_LOC)
        logits[bs] = r["logits_o"].reshape(B_LOC, 10)
        dcaps[bs] = r["digit_o"].reshape(B_LOC, 10, 16)
        recon[bs] = r["recon_o"]
        pcaps[bs] = r["caps_o"].transpose(2, 1, 0, 3).reshape(B_LOC, NCAPS, 8)
        c_full[bs] = r["c_o"].transpose(2, 3, 1, 0).reshape(
            B_LOC, 10, NCAPS, 1)
        b_full[bs] = r["b_o"].transpose(2, 3, 1, 0).reshape(
            B_LOC, 10, NCAPS, 1)
    return logits, recon, pcaps, dcaps, c_full, b_full


# ---------------- PJRT SPMD execution (axon) ----------------

class SpmdRunner:
    def __init__(self, nc, n_cores):
        import jax
        from jax.sharding import Mesh, PartitionSpec
        from jax.experimental.shard_map import shard_map
        from concourse.bass2jax import (
            _bass_exec_p, install_neuronx_cc_hook, partition_id_tensor)

        install_neuronx_cc_hook()
        self.jax = jax
        self.nc = nc
        self.n_cores = n_cores
        partition_name = (nc.partition_id_tensor.name
                          if nc.partition_id_tensor else None)
        in_names, out_names, out_avals, zero_outs = [], [], [], []
        for alloc in nc.m.functions[0].allocations:
            if not isinstance(alloc, mybir.MemoryLocationSet):
                continue
            name = alloc.memorylocations[0].name
            if alloc.kind == "ExternalInput":
                if name != partition_name:
                    in_names.append(name)
            elif alloc.kind == "ExternalOutput":
                shape = tuple(alloc.tensor_shape)
                dtype = mybir.dt.np(alloc.dtype)
                out_names.append(name)
                out_avals.append(jax.core.ShapedArray(shape, dtype))
                zero_outs.append(np.zeros(shape, dtype))
        self.in_names, self.out_names = in_names, out_names
        self.out_avals, self.zero_outs = out_avals, zero_outs
        n_params, n_outs = len(in_names), len(out_avals)
        all_in_names = list(in_names) + list(out_names)
        if partition_name is not None:
            all_in_names.append(partition_name)
        donate = tuple(range(n_params, n_params + n_outs))

        def _body(*args):
            operands = list(args)
            if partition_name is not None:
                operands.append(partition_id_tensor())
            outs = _bass_exec_p.bind(
                *operands,
                out_avals=tuple(out_avals),
                in_names=tuple(all_in_names),
                out_names=tuple(out_names),
                lowering_input_output_aliases=(),
                sim_require_finite=True,
                sim_require_nnan=True,
                nc=nc,
            )
            return tuple(outs)

        devices = jax.devices()[:n_cores]
        assert len(devices) >= 1
        self.mesh = Mesh(np.asarray(devices), ("core",))
        in_specs = (PartitionSpec("core"),) * (n_params + n_outs)
        out_specs = (PartitionSpec("core"),) * n_outs
        self.sharded = jax.jit(
            shard_map(_body, mesh=self.mesh, in_specs=in_specs,
                      out_specs=out_specs, check_rep=False),
            donate_argnums=donate, keep_unused=True,
        )

    def stage(self, in_maps):
        from jax.sharding import NamedSharding, PartitionSpec
        n = self.n_cores
        per_core = [[np.asarray(m[name]) for name in self.in_names]
                    for m in in_maps]
        concat_in = [
            np.concatenate([per_core[c][i] for c in range(n)], axis=0)
            for i in range(len(self.in_names))
        ]
        sharding = NamedSharding(self.mesh, PartitionSpec("core"))
        dev_in = [self.jax.device_put(a, sharding) for a in concat_in]
        self.jax.block_until_ready(dev_in)
        return dev_in

    def run(self, in_maps=None, reps=1, dev_in=None):
        jax = self.jax
        n = self.n_cores
        if dev_in is None:
            dev_in = self.stage(in_maps)
        times, out_arrs = [], None
        for _ in range(reps):
            concat_zeros = [
                np.zeros((n * z.shape[0], *z.shape[1:]), z.dtype)
                for z in self.zero_outs
            ]
            t0 = time.perf_counter()
            out_arrs = self.sharded(*dev_in, *concat_zeros)
            jax.block_until_ready(out_arrs)
            times.append(time.perf_counter() - t0)
        results = [
            {
                name: np.asarray(out_arrs[i]).reshape(
                    n, *self.out_avals[i].shape)[c]
                for i, name in enumerate(self.out_names)
            }
            for c in range(n)
        ]
        return results, times


_RUNNER = None


def get_runner():
    global _RUNNER
    if _RUNNER is None:
        nc = build_program()
        _RUNNER = SpmdRunner(nc, N_CORES)
    return _RUNNER


def kernel(**inputs):
    inputs = {k: np.asarray(v) for k, v in inputs.items()}
    in_maps = prep_inputs(**inputs)
    runner = get_runner()
    results, _ = runner.run(in_maps, reps=1)
    return assemble(results)
